# revision 1
# baseline (speedup 1.0000x reference)
"""SMEAR MoE layer (nn_MoELayer_SMEAR) Trainium2 Bass kernel.

Problem: B=8, L=2048, D=1024, H=4096, E=8, fp32.
  logits = x @ router_w.T + router_b; probs = softmax(logits) * mask
  up = probs.sum(L) / clip(mask.sum(L), 1)            # [B, E]
  mW1 = up @ W1 ; mb1 = up @ b1 ; mW2 = up @ W2 ; mb2 = up @ b2  (merged per b)
  out = relu(x @ mW1.T + mb1) @ mW2.T + mb2

Sharding (8 cores): dp=2 over B x tp=4 over H.
  core c: group g=c//4 handles batches g*4..g*4+3; rank r=c%4 handles
  H-shard [r*1024,(r+1)*1024). Each core computes partial outputs for its
  4 batches over its H-shard; host sums the 4 partials per group and
  transposes ([o,t] -> [t,o]) to unshard.

Device phases per core:
  B) router: logits^T per 512-token chunk (PE, f32r), +bias, PE-transpose
     to [t,128 x e], exp (ACT), row-sum+recip (DVE), up-accumulate (PE).
  C) merge: mW1T[b] = sum_e up[b,e]*W1T[e] via DVE fused scalar_tensor_tensor,
     streamed W chunks, merged weights written to DRAM as float32r.
  D) MLP per b: hiddenT = relu(mW1T^T x^T + b) [PE f32r, ACT relu copyback],
     outT = mW2T^T hiddenT + b2 (owner core only adds b2), partials to DRAM.
"""

import numpy as np
import ml_dtypes

import concourse.bass as bass
import concourse.bacc as bacc
import concourse.mybir as mybir
import concourse.tile as tile
from concourse.bass_utils import run_bass_kernel_spmd
from concourse.masks import make_identity

P = 128
B, L, D, H, E = 8, 2048, 1024, 4096, 8
NB = 4          # batches per core
HS = H // 4     # h-shard width per core
DS = D // P     # 8 d-subtiles
HSUB = HS // P  # 8 h-subtiles in shard
OSUB = D // P   # 8 output subtiles
TCH = 512       # moving-dim chunk for matmuls
TC = L // TCH   # 4 chunks per batch

F32 = mybir.dt.float32
F32R = mybir.dt.float32r
BF16 = mybir.dt.bfloat16
AF = mybir.ActivationFunctionType
ALU = mybir.AluOpType
AX = mybir.AxisListType

_CACHED_NC = None


def _build():
    nc = bacc.Bacc("TRN2", target_bir_lowering=False, debug=False)

    xT = nc.dram_tensor("xT", [NB, D, L], F32R, kind="ExternalInput")
    xTb = nc.dram_tensor("xTb", [NB, D, L], BF16, kind="ExternalInput")
    maskT = nc.dram_tensor("maskT", [L, NB], F32, kind="ExternalInput")
    rwT = nc.dram_tensor("rwT", [D, E], BF16, kind="ExternalInput")
    rb = nc.dram_tensor("rb", [E, 1], F32, kind="ExternalInput")
    W1T = nc.dram_tensor("W1T", [E, D, HS], F32R, kind="ExternalInput")
    W2T = nc.dram_tensor("W2T", [E, HS, D], F32, kind="ExternalInput")
    b1T = nc.dram_tensor("b1T", [HS, E], F32, kind="ExternalInput")
    b2T = nc.dram_tensor("b2T", [D, E], F32, kind="ExternalInput")
    ownc = nc.dram_tensor("ownc", [NB, 1], F32, kind="ExternalInput")
    outp = nc.dram_tensor("outp", [NB, D, L], F32, kind="ExternalOutput")

    mW1d = nc.dram_tensor("mW1d", [NB, D, HS], F32R)
    mW2d = nc.dram_tensor("mW2d", [NB, HS, D], F32R)

    with tile.TileContext(nc) as tc:
        with tc.tile_pool(name="const", bufs=1) as const:
            ident = const.tile([P, P], F32)
            make_identity(nc, ident)
            ones_col = const.tile([P, 1], F32)
            nc.gpsimd.memset(ones_col[:], 1.0)
            ones_row = const.tile([1, P], F32)
            nc.gpsimd.memset(ones_row[:], 1.0)

            rwT_sb = const.tile([P, DS, E], BF16)
            nc.sync.dma_start(rwT_sb[:], rwT.ap().rearrange("(s p) e -> p s e", p=P))
            rb_sb = const.tile([E, 1], F32)
            nc.sync.dma_start(rb_sb[:], rb.ap())
            maskT_sb = const.tile([P, L // P, NB], F32)
            nc.sync.dma_start(maskT_sb[:], maskT.ap().rearrange("(q p) b -> p q b", p=P))
            b1T_sb = const.tile([P, HSUB, E], F32)
            nc.sync.dma_start(b1T_sb[:], b1T.ap().rearrange("(s p) e -> p s e", p=P))
            b2T_sb = const.tile([P, OSUB, E], F32)
            nc.sync.dma_start(b2T_sb[:], b2T.ap().rearrange("(s p) e -> p s e", p=P))
            own_sb = const.tile([NB, 1], F32)
            nc.sync.dma_start(own_sb[:], ownc.ap())

            up_sb = const.tile([E, NB], F32)
            upT_sb = const.tile([NB, E], F32)
            upTo_sb = const.tile([NB, E], F32)
            up_bc = const.tile([P, NB, E], F32)
            upo_bc = const.tile([P, NB, E], F32)
            mb1_sb = const.tile([P, NB, HSUB], F32)
            mb2_sb = const.tile([P, NB, OSUB], F32)
            invbc_sb = const.tile([P, NB], F32)

            # ---------------- Phase B: router ----------------
            with tc.tile_pool(name="rpsum", bufs=1, space="PSUM") as rpsum, \
                 tc.tile_pool(name="rsb", bufs=6) as rsb, \
                 tc.tile_pool(name="xrt", bufs=4) as xrt, \
                 tc.tile_pool(name="lgp", bufs=2, space="PSUM") as lgp, \
                 tc.tile_pool(name="trp", bufs=3, space="PSUM") as trp, \
                 tc.tile_pool(name="upp", bufs=2, space="PSUM") as upp:

                # denominators: denom[b] = clip(sum_t mask, 1); invbc = 1/denom bcast
                mpart = rsb.tile([P, NB], F32)
                for b in range(NB):
                    nc.vector.tensor_reduce(
                        mpart[:, b:b + 1], maskT_sb[:, :, b], axis=AX.X, op=ALU.add)
                den_ps = rpsum.tile([NB, 1], F32, tag="rps")
                nc.tensor.matmul(den_ps[:], mpart[:], ones_col[:], start=True, stop=True)
                den_sb = rsb.tile([NB, 1], F32)
                nc.vector.tensor_scalar_max(den_sb[:], den_ps[:], 1.0)
                inv_sb = rsb.tile([NB, 1], F32)
                nc.vector.reciprocal(inv_sb[:], den_sb[:])
                invT_ps = rpsum.tile([1, NB], F32, tag="rps")
                nc.tensor.transpose(invT_ps[:], inv_sb[:], ident[:NB, :NB])
                invT_sb = rsb.tile([1, NB], F32)
                nc.vector.tensor_copy(invT_sb[:], invT_ps[:])
                invbc_ps = rpsum.tile([P, NB], F32, tag="rps")
                nc.tensor.matmul(invbc_ps[:], ones_row[:], invT_sb[:], start=True, stop=True)
                nc.vector.tensor_copy(invbc_sb[:], invbc_ps[:])

                NQ = TCH // P  # 4 transpose sub-chunks per 512 chunk
                for b in range(NB):
                    # maskS = mask * inv_denom for this b (free-dim broadcast)
                    maskS = rsb.tile([P, L // P], F32, tag="maskS")
                    nc.vector.tensor_tensor(
                        maskS[:], maskT_sb[:, :, b],
                        invbc_sb[:, b:b + 1].to_broadcast((P, L // P)), ALU.mult)
                    up_ps = upp.tile([E, 1], F32)
                    for t4 in range(TC):
                        xt = xrt.tile([P, DS, TCH], BF16, tag="xrt")
                        nc.sync.dma_start(
                            xt[:],
                            xTb.ap()[b].rearrange("(s p) t -> p s t", p=P)[
                                :, :, t4 * TCH:(t4 + 1) * TCH])
                        lg_ps = lgp.tile([E, TCH], F32)
                        for dsb in range(DS):
                            nc.tensor.matmul(lg_ps[:], rwT_sb[:, dsb], xt[:, dsb],
                                             start=(dsb == 0), stop=(dsb == DS - 1))
                        lgT = rsb.tile([E, TCH], F32, tag="lgT")
                        nc.scalar.activation(lgT[:], lg_ps[:], AF.Identity, bias=rb_sb[:])
                        # 4 transposes into one psum tile [P, 4*E]
                        tr_ps = trp.tile([P, NQ * E], F32)
                        for q in range(NQ):
                            nc.tensor.matmul(
                                tr_ps[:, q * E:(q + 1) * E],
                                lgT[:, q * P:(q + 1) * P], ident[:E, :E],
                                is_transpose=True,
                                start=(q == 0), stop=(q == NQ - 1))
                        pexp = rsb.tile([P, NQ, E], F32, tag="pexp")
                        nc.scalar.activation(pexp[:], tr_ps[:], AF.Exp)
                        s4 = rsb.tile([P, NQ], F32, tag="s4")
                        nc.vector.tensor_reduce(s4[:], pexp[:], axis=AX.X, op=ALU.add)
                        sr4 = rsb.tile([P, NQ], F32, tag="sr4")
                        nc.vector.reciprocal(sr4[:], s4[:])
                        r4 = rsb.tile([P, NQ], F32, tag="r4")
                        nc.vector.tensor_tensor(
                            r4[:], sr4[:], maskS[:, t4 * NQ:(t4 + 1) * NQ], ALU.mult)
                        for q in range(NQ):
                            nc.tensor.matmul(
                                up_ps[:], pexp[:, q], r4[:, q:q + 1],
                                start=(t4 == 0 and q == 0),
                                stop=(t4 == TC - 1 and q == NQ - 1))
                    nc.vector.tensor_copy(up_sb[:, b:b + 1], up_ps[:])

                # broadcast up across partitions; owner-masked copy for b2
                upT_ps = rpsum.tile([NB, E], F32, tag="rps")
                nc.tensor.transpose(upT_ps[:], up_sb[:], ident[:E, :E])
                nc.vector.tensor_copy(upT_sb[:], upT_ps[:])
                nc.vector.tensor_scalar_mul(upTo_sb[:], upT_sb[:], own_sb[:])
                for b in range(NB):
                    rowu = rsb.tile([1, E], F32, tag="rowu")
                    nc.sync.dma_start(rowu[:], upT_sb[b:b + 1, :])
                    rowo = rsb.tile([1, E], F32, tag="rowo")
                    nc.sync.dma_start(rowo[:], upTo_sb[b:b + 1, :])
                    bc_ps = rpsum.tile([P, E], F32, tag="rps")
                    nc.tensor.matmul(bc_ps[:], ones_row[:], rowu[:], start=True, stop=True)
                    nc.vector.tensor_copy(up_bc[:, b], bc_ps[:])
                    bo_ps = rpsum.tile([P, E], F32, tag="rps")
                    nc.tensor.matmul(bo_ps[:], ones_row[:], rowo[:], start=True, stop=True)
                    nc.vector.tensor_copy(upo_bc[:, b], bo_ps[:])

                # merged biases: mb1[b] = sum_e up[b,e] b1T[:,e]; mb2 owner-masked
                for b in range(NB):
                    nc.vector.tensor_scalar_mul(
                        mb1_sb[:, b], b1T_sb[:, :, 0], up_bc[:, b, 0:1])
                    nc.vector.tensor_scalar_mul(
                        mb2_sb[:, b], b2T_sb[:, :, 0], upo_bc[:, b, 0:1])
                    for e in range(1, E):
                        nc.vector.scalar_tensor_tensor(
                            mb1_sb[:, b], b1T_sb[:, :, e], up_bc[:, b, e:e + 1],
                            mb1_sb[:, b], ALU.mult, ALU.add)
                        nc.vector.scalar_tensor_tensor(
                            mb2_sb[:, b], b2T_sb[:, :, e], upo_bc[:, b, e:e + 1],
                            mb2_sb[:, b], ALU.mult, ALU.add)

            # ---------------- Phase C: weight merge ----------------
            # scaled identities for the PE-side W1 merge
            upid = const.tile([P, NB, E, P], F32R)
            for b in range(NB):
                for e in range(E):
                    nc.vector.tensor_scalar_mul(
                        upid[:, b, e], ident[:], up_bc[:, b, e:e + 1])

            # W1 merged on PE (scaled-identity matmuls, ready early for L1);
            # W2 merged on DVE (fused scalar_tensor_tensor) concurrently.
            with tc.tile_pool(name="w1ch", bufs=2) as w1ch, \
                 tc.tile_pool(name="wch", bufs=2) as wch, \
                 tc.tile_pool(name="accp", bufs=3) as accp, \
                 tc.tile_pool(name="mrgps", bufs=6, space="PSUM") as mrgps, \
                 tc.tile_pool(name="moutp", bufs=6) as moutp:
                for dsb in range(DS):
                    # --- W1 chunk -> PE merge ---
                    c1 = w1ch.tile([P, E, HS], F32R, tag="w1ch")
                    nc.sync.dma_start(
                        c1[:],
                        W1T.ap()[:, dsb * P:(dsb + 1) * P, :].rearrange(
                            "e p h -> p e h"))
                    for b in range(NB):
                        for hb in range(HS // TCH):
                            ps = mrgps.tile([P, TCH], F32, tag="mps")
                            for e in range(E):
                                nc.tensor.matmul(
                                    ps[:], upid[:, b, e],
                                    c1[:, e, hb * TCH:(hb + 1) * TCH],
                                    start=(e == 0), stop=(e == E - 1))
                            mo = moutp.tile([P, TCH], F32R, tag="mo")
                            nc.scalar.activation(mo[:], ps[:], AF.Identity)
                            nc.sync.dma_start(
                                mW1d.ap()[b, dsb * P:(dsb + 1) * P,
                                          hb * TCH:(hb + 1) * TCH], mo[:])
                    # --- W2 chunk -> DVE merge ---
                    c2 = wch.tile([P, E, D], F32, tag="wch")
                    nc.sync.dma_start(
                        c2[:],
                        W2T.ap()[:, dsb * P:(dsb + 1) * P, :].rearrange(
                            "e p h -> p e h"))
                    for b in range(NB):
                        acc = accp.tile([P, D], F32, tag="acc")
                        nc.vector.tensor_scalar_mul(
                            acc[:], c2[:, 0], up_bc[:, b, 0:1])
                        for e in range(1, E - 1):
                            nc.vector.scalar_tensor_tensor(
                                acc[:], c2[:, e], up_bc[:, b, e:e + 1],
                                acc[:], ALU.mult, ALU.add)
                        accr = accp.tile([P, D], F32R, tag="accr")
                        nc.vector.scalar_tensor_tensor(
                            accr[:], c2[:, E - 1], up_bc[:, b, E - 1:E],
                            acc[:], ALU.mult, ALU.add)
                        nc.sync.dma_start(
                            mW2d.ap()[b, dsb * P:(dsb + 1) * P, :], accr[:])

            # ---------------- Phase D: MLP ----------------
            with tc.tile_pool(name="xtb", bufs=1) as xtbp, \
                 tc.tile_pool(name="hid", bufs=1) as hidp, \
                 tc.tile_pool(name="wmm", bufs=4) as wmmp, \
                 tc.tile_pool(name="osb", bufs=6) as osbp, \
                 tc.tile_pool(name="mmp", bufs=6, space="PSUM") as mmp:
                for b in range(NB):
                    xtb = xtbp.tile([P, DS, L], F32R, tag="xtb")
                    for t4 in range(TC):
                        nc.sync.dma_start(
                            xtb[:, :, t4 * TCH:(t4 + 1) * TCH],
                            xT.ap()[b].rearrange("(s p) t -> p s t", p=P)[
                                :, :, t4 * TCH:(t4 + 1) * TCH])
                    hid = hidp.tile([P, HSUB, L], F32R, tag="hid")
                    for hb in range(HSUB):
                        w1t = wmmp.tile([P, DS, P], F32R, tag="w1t")
                        nc.sync.dma_start(
                            w1t[:],
                            mW1d.ap()[b].rearrange("(s p) h -> p s h", p=P)[
                                :, :, hb * P:(hb + 1) * P])
                        for t4 in range(TC):
                            ps = mmp.tile([P, TCH], F32, tag="ps")
                            for dsb in range(DS):
                                nc.tensor.matmul(
                                    ps[:], w1t[:, dsb],
                                    xtb[:, dsb, t4 * TCH:(t4 + 1) * TCH],
                                    start=(dsb == 0), stop=(dsb == DS - 1))
                            nc.scalar.activation(
                                hid[:, hb, t4 * TCH:(t4 + 1) * TCH], ps[:],
                                AF.Relu, bias=mb1_sb[:, b, hb:hb + 1])
                    for ob in range(OSUB):
                        w2t = wmmp.tile([P, HSUB, P], F32R, tag="w2t")
                        nc.sync.dma_start(
                            w2t[:],
                            mW2d.ap()[b].rearrange("(s p) o -> p s o", p=P)[
                                :, :, ob * P:(ob + 1) * P])
                        for t4 in range(TC):
                            ps = mmp.tile([P, TCH], F32, tag="ps")
                            for hs in range(HSUB):
                                nc.tensor.matmul(
                                    ps[:], w2t[:, hs],
                                    hid[:, hs, t4 * TCH:(t4 + 1) * TCH],
                                    start=(hs == 0), stop=(hs == HSUB - 1))
                            ot = osbp.tile([P, TCH], F32, tag="ot")
                            nc.vector.tensor_scalar_add(
                                ot[:], ps[:], mb2_sb[:, b, ob:ob + 1])
                            nc.sync.dma_start(
                                outp.ap()[b, ob * P:(ob + 1) * P,
                                          t4 * TCH:(t4 + 1) * TCH], ot[:])

    nc.compile()
    return nc


def _get_nc():
    global _CACHED_NC
    if _CACHED_NC is None:
        _CACHED_NC = _build()
    return _CACHED_NC


def kernel(x, mask, router_w, router_b, W1, b1, W2, b2, _trace=False):
    x = np.asarray(x, np.float32)
    mask = np.asarray(mask, np.float32)
    router_w = np.asarray(router_w, np.float32)
    router_b = np.asarray(router_b, np.float32)
    W1 = np.asarray(W1, np.float32)
    b1 = np.asarray(b1, np.float32)
    W2 = np.asarray(W2, np.float32)
    b2 = np.asarray(b2, np.float32)

    nc = _get_nc()

    # host-side layout prep (sharding): transposes only, no reductions
    xT_all = np.ascontiguousarray(x.transpose(0, 2, 1))        # [B, D, L]
    xTb_all = xT_all.astype(ml_dtypes.bfloat16)
    W1T_all = np.ascontiguousarray(W1.transpose(0, 2, 1))      # [E, D, H]
    W2T_all = np.ascontiguousarray(W2.transpose(0, 2, 1))      # [E, H, D]
    rwT = np.ascontiguousarray(router_w.T).astype(ml_dtypes.bfloat16)  # [D, E]
    rbc = np.ascontiguousarray(router_b.reshape(E, 1))
    b1T_full = np.ascontiguousarray(b1.T)                      # [H, E]
    b2T = np.ascontiguousarray(b2.T)                           # [D, E]

    in_maps = []
    for c in range(8):
        g, r = c // 4, c % 4
        hs = slice(r * HS, (r + 1) * HS)
        own = np.zeros((NB, 1), np.float32)
        own[r, 0] = 1.0
        in_maps.append({
            "xT": xT_all[g * NB:(g + 1) * NB],
            "xTb": xTb_all[g * NB:(g + 1) * NB],
            "maskT": np.ascontiguousarray(mask[g * NB:(g + 1) * NB].T),
            "rwT": rwT,
            "rb": rbc,
            "W1T": np.ascontiguousarray(W1T_all[:, :, hs]),
            "W2T": np.ascontiguousarray(W2T_all[:, hs, :]),
            "b1T": np.ascontiguousarray(b1T_full[hs]),
            "b2T": b2T,
            "ownc": own,
        })

    res = run_bass_kernel_spmd(nc, in_maps, core_ids=list(range(8)),
                               trace=_trace)

    out = np.empty((B, L, D), np.float32)
    for g in range(2):
        acc = res.results[g * 4]["outp"].copy()
        for r in range(1, 4):
            acc += res.results[g * 4 + r]["outp"]
        for j in range(NB):
            out[g * NB + j] = acc[j].T
    if _trace:
        return out, res
    return out

